# revision 56
# baseline (speedup 1.0000x reference)
"""Trainium2 Bass kernel for the 2D Gaussian splatting model (nn_GaussianModel2D).

Math (per pixel p, gaussians n = 0..255 in order):
    e_n(p)   = -(a dx^2 + 2b dx dy + c dy^2) + ln(opac_n)      (quadratic in x,y)
    alpha_n  = exp(e_n)            (clip to 0.99 never binds for this input;
                                    checked on host, fallback applies it)
    u_n      = 1 - alpha_n
    T_m      = prod_{k<m} u_k ;   scan_t = T_{t+1}  (inclusive cumprod)
    out_c    = clip(c0_c + sum_t gamma_{t,c} * scan_t, 0, 1)
    where gamma_{t,c} = col_{t+1,c} - col_{t,c} (t<255), gamma_{255,c} = 1 - col_{255,c}
    (Abel summation of sum_n w_n col_n + bg;  w_n = T_n - T_{n+1}, bg = T_256)

Device layout per core (1/8 of the image rows = 32768 pixels):
    - exponent: PE matmul, K=6 features [x'^2, x'y', y'^2, x', y', 1] (centered),
      fp32r, out (128 pix, 256 g) chunks in PSUM
    - alpha: ACT exp, PSUM->SBUF, batched 4 chunks per call
    - u = 1 - alpha: DVE tensor_scalar
    - scan_t: DVE tensor_tensor_scan (cumprod along free axis), fp16 out
    - transpose scan (pix, g) -> (g, pix): DMA xbar transpose, fp16
    - rendered: PE matmul gamma^T @ scan^T accumulated in PSUM at partition
      offsets 32m (zero-padded gamma to 32 cols so PSUM is fully written)
    - final: DVE tensor_scalar (+c0, min 1) PSUM->SBUF, DMA out (3, 32768)
"""

import numpy as np

H, W, N = 512, 512, 256
NCORES = 8
ROWS_PER_CORE = H // NCORES            # 64
PIX = ROWS_PER_CORE * W                # 32768 pixels per core
CHUNK = 128                            # pixels per matmul chunk
NCHUNK = PIX // CHUNK                  # 256
BATCH = 4                              # chunks per ACT/u batch (psum tile 128x1024)
NBATCH = NCHUNK // BATCH               # 64
GROUP = 16                             # chunks per psum-out group
NGROUP = NCHUNK // GROUP               # 16

_CACHE = {}


def _build_program(apply_opacity_clip: bool):
    import concourse.bass as bass
    import concourse.bacc as bacc
    import concourse.tile as tile
    import concourse.mybir as mybir
    from contextlib import ExitStack

    fp32 = mybir.dt.float32
    fp32r = mybir.dt.float32r
    fp16 = mybir.dt.float16
    Alu = mybir.AluOpType
    Act = mybir.ActivationFunctionType

    nc = bacc.Bacc("TRN2", target_bir_lowering=False, debug=False,
                   num_devices=NCORES)

    ft_d = nc.dram_tensor("ft", [18, PIX], fp16, kind="ExternalInput")
    c6_d = nc.dram_tensor("c6", [18, N], fp16, kind="ExternalInput")
    gam_d = nc.dram_tensor("gam", [N, 32], fp16, kind="ExternalInput")
    c0_d = nc.dram_tensor("c0", [128, 1], fp32, kind="ExternalInput")
    out_d = nc.dram_tensor("out", [3, PIX], fp32, kind="ExternalOutput")

    with tile.TileContext(nc) as tc, ExitStack() as ctx:
        consts = ctx.enter_context(tc.tile_pool(name="consts", bufs=1))
        apool = ctx.enter_context(tc.tile_pool(name="alpha", bufs=2))
        upool = ctx.enter_context(tc.tile_pool(name="u", bufs=2))
        tpool = ctx.enter_context(tc.tile_pool(name="tsc", bufs=3))
        ttpool = ctx.enter_context(tc.tile_pool(name="tt", bufs=3))
        opool = ctx.enter_context(tc.tile_pool(name="osb", bufs=2))
        eps_pool = ctx.enter_context(tc.tile_pool(name="eps", bufs=2, space="PSUM"))
        rps_pool = ctx.enter_context(tc.tile_pool(name="rps", bufs=2, space="PSUM"))

        ft_sb = consts.tile([18, PIX], fp16)
        c6_sb = consts.tile([18, N], fp16)
        gam_sb = consts.tile([128, 2, 32], fp16)
        c0_sb = consts.tile([128, 1], fp32)
        nc.sync.dma_start(ft_sb[:], ft_d[:])
        nc.sync.dma_start(c6_sb[:], c6_d[:])
        nc.sync.dma_start(gam_sb[:], gam_d.ap().rearrange("(b k) c -> k b c", k=128))
        nc.sync.dma_start(c0_sb[:], c0_d[:])

        for g in range(NGROUP):            # 16 groups of 16 chunks
            r_ps = rps_pool.tile([128, 512], fp32)
            for m in range(GROUP // BATCH):  # 4 psum partition-groups
                jbase = g * GROUP + m * BATCH
                e_ps = eps_pool.tile([128, BATCH * N], fp32)
                for q in range(BATCH):
                    j = jbase + q
                    nc.tensor.matmul(
                        e_ps[:, q * N:(q + 1) * N],
                        lhsT=ft_sb[:, j * CHUNK:(j + 1) * CHUNK],
                        rhs=c6_sb[:],
                        start=True, stop=True)
                al = apool.tile([128, BATCH * N], fp32)
                nc.scalar.activation(al[:], e_ps[:], Act.Exp)
                u = upool.tile([128, BATCH * N], fp32)
                if apply_opacity_clip:
                    nc.vector.tensor_scalar(al[:], al[:], 0.99, None, Alu.min)
                nc.vector.tensor_scalar(u[:], al[:], -1.0, 1.0, Alu.mult, Alu.add)
                for q in range(BATCH):
                    t_sc = tpool.tile([128, N], fp16)
                    nc.vector.tensor_tensor_scan(
                        t_sc[:], data0=u[:, q * N:(q + 1) * N],
                        data1=u[:, q * N:(q + 1) * N], initial=1.0,
                        op0=Alu.mult, op1=Alu.bypass)
                    tt = ttpool.tile([128, 2 * CHUNK], fp16)
                    for b in range(2):
                        nc.sync.dma_start_transpose(
                            tt[:, b * CHUNK:(b + 1) * CHUNK],
                            t_sc[:, b * 128:(b + 1) * 128])
                    for b in range(2):
                        nc.tensor.matmul(
                            r_ps[32 * m:32 * m + 32, q * CHUNK:(q + 1) * CHUNK],
                            lhsT=gam_sb[:, b, :],
                            rhs=tt[:, b * CHUNK:(b + 1) * CHUNK],
                            start=(b == 0), stop=(b == 1),
                            tile_position=(0, 32 * m))
            o_sb = opool.tile([128, 512], fp32)
            nc.vector.tensor_scalar(o_sb[:], r_ps[:], c0_sb[:], 1.0,
                                    Alu.add, Alu.min)
            for m in range(4):
                nc.sync.dma_start(
                    out_d.ap()[:, (g * GROUP + m * BATCH) * CHUNK:
                               (g * GROUP + (m + 1) * BATCH) * CHUNK],
                    o_sb[32 * m:32 * m + 3, :])
    nc.compile()
    return nc


def _prep_host(coords, means, log_scales, rotations, raw_colors, raw_opacities):
    """Tiny host-side parameter preparation (float64 for coefficient accuracy)."""
    f64 = np.float64
    scales = np.exp(log_scales.astype(f64))
    sx2, sy2 = scales[:, 0] ** 2, scales[:, 1] ** 2
    cos_r = np.cos(rotations.astype(f64))
    sin_r = np.sin(rotations.astype(f64))
    a = cos_r ** 2 / (2 * sx2) + sin_r ** 2 / (2 * sy2)
    b = -sin_r * cos_r / (2 * sx2) + sin_r * cos_r / (2 * sy2)
    c = sin_r ** 2 / (2 * sx2) + cos_r ** 2 / (2 * sy2)
    opac = 1.0 / (1.0 + np.exp(-raw_opacities.astype(f64)))
    colors = 1.0 / (1.0 + np.exp(-raw_colors.astype(f64)))   # (N, 3)

    mx = means[:, 0].astype(f64) - 0.5
    my = means[:, 1].astype(f64) - 0.5
    # e = -(a dx^2 + 2 b dx dy + c dy^2) + ln(opac), expanded over centered
    # features [x^2, xy, y^2, x, y, 1]
    C6 = np.stack([
        -a,
        -2.0 * b,
        -c,
        2.0 * a * mx + 2.0 * b * my,
        2.0 * b * mx + 2.0 * c * my,
        -(a * mx ** 2 + 2.0 * b * mx * my + c * my ** 2) + np.log(opac),
    ])                                                        # (6, N) f64
    c_hi = C6.astype(np.float16)
    c_lo = (C6 - c_hi.astype(f64)).astype(np.float16)
    C6 = np.concatenate([c_hi, c_hi, c_lo])                  # (18, N)

    gam = np.zeros((N, 32), np.float64)
    gam[:N - 1, :3] = colors[1:] - colors[:-1]
    gam[N - 1, :3] = 1.0 - colors[N - 1]
    gam = gam.astype(np.float16)

    c0 = np.zeros((128, 1), np.float32)
    for m in range(4):
        c0[32 * m:32 * m + 3, 0] = colors[0].astype(np.float32)

    x = coords[:, :, 0].astype(f64) - 0.5                    # (H, W)
    y = coords[:, :, 1].astype(f64) - 0.5
    feats = np.stack([x * x, x * y, y * y, x, y, np.ones_like(x)])  # (6, H, W)
    feats = feats.reshape(6, NCORES, PIX)
    f_hi = feats.astype(np.float16)
    f_lo = (feats - f_hi.astype(f64)).astype(np.float16)
    feats = np.concatenate([f_hi, f_lo, f_hi])               # (18, NCORES, PIX)

    clip_needed = bool((opac > 0.99).any())
    return feats, C6, gam, c0, clip_needed


def kernel(coords, means, log_scales, rotations, raw_colors, raw_opacities):
    from concourse.bass_utils import run_bass_kernel_spmd

    feats, C6, gam, c0, clip_needed = _prep_host(
        coords, means, log_scales, rotations, raw_colors, raw_opacities)

    key = ("prog", clip_needed)
    if key not in _CACHE:
        _CACHE[key] = _build_program(clip_needed)
    nc = _CACHE[key]

    in_maps = [
        {"ft": np.ascontiguousarray(feats[:, k]), "c6": C6, "gam": gam, "c0": c0}
        for k in range(NCORES)
    ]
    res = run_bass_kernel_spmd(nc, in_maps, list(range(NCORES)))
    out = np.stack([res.results[k]["out"] for k in range(NCORES)])  # (8, 3, PIX)
    out = out.reshape(NCORES, 3, ROWS_PER_CORE, W).transpose(0, 2, 3, 1)
    out = out.reshape(H, W, 3)
    return np.clip(out, 0.0, 1.0).astype(np.float32)
